# revision 22
# baseline (speedup 1.0000x reference)
"""Trainium2 Bass kernel for nn_Attention_Weighted_Context_Generation.

ctx = A @ F,  A = weights.reshape(9216, 9216),
F = cnn_feature.reshape(256, 9216).T; returns ctx.reshape(9216,1,1,256).

fp8 e4m3 scheme (host-sim 1.79e-2 rel err vs the 2e-2 gate; deterministic
host quantization, fp32 PSUM accumulation):
  A = 0.5 + u,  u in [-0.5, 0.5) -> e4m3   (0.5*colsum(F) rank-1 term
                                            added exactly on host)
  F -> e4m3 (single plane)
  all 72 k-tiles as 36 true DoubleRow pairs -> 216 matmul passes
  ctx = raw/(s_u*s_F) + 0.5*colsum(F)      (host dequant; raw stored bf16)

v2 changes vs the compensated baseline (63.8us):
  - no compensation region: 216 passes (35.0us stream floor) vs 252
  - DRAM images pre-packed partition-major [128, 72*1408]: every batch
    DMA is a [128, X]->[128, X] 2D copy with nt*1408-byte contiguous
    per-partition runs (5.6-8.4 KB packets vs 1.4 KB before; the DGE's
    ~10ns/packet overhead amortizes, 16 engines x ~27 GB/s)
  - whole 99 KB/partition stream is SBUF-resident: no ring reuse, no
    PE->DMA backpressure semaphore; sync issues all 14 batches
    back-to-back so the DGE never idles between batches

Sharding: rows of A across 8 cores (1152 each), F replicated. Flipped
layout (F stationary): 6 PSUM chains = 2 c-chunks x 3 m-chunks of 384;
out is ctx^T [256, 1152] accumulated over all 72 k-tiles.
"""

import numpy as np

import concourse.bass as bass
from concourse import mybir
from concourse.bass_utils import run_bass_kernel_spmd

N_CORES = 8
HW = 9216
C = 256
M_PER = HW // N_CORES   # 1152
KT = HW // 128          # 72 k-tiles
WU = M_PER + C          # 1408 bytes/tile/partition: u8T | F8
# batch layout in tiles (even so DoubleRow pairs never straddle):
# small first batch so the PE can start while the DGE is streaming.
BATCH = [2, 2, 2, 4, 4, 4, 4, 4, 6, 8, 8, 8, 8, 8]
assert sum(BATCH) == KT
NB = len(BATCH)
NSEM = 8
MCH = 384
NDUMMY = 11             # p-state warm-up matmuls into PSUM bank 6
E4 = mybir.dt.float8e4
DR = mybir.MatmulPerfMode.DoubleRow

_TSTART = [sum(BATCH[:i]) for i in range(NB)]


def build_bass():
    nc = bass.Bass("TRN2", target_bir_lowering=False, debug=False,
                   num_devices=N_CORES)
    atf = nc.dram_tensor("atf", [128, KT * WU], E4,
                         kind="ExternalInput").ap()
    out = nc.dram_tensor("out", [C, M_PER], mybir.dt.bfloat16,
                         kind="ExternalOutput").ap()

    from contextlib import ExitStack
    with (
        ExitStack() as stack,
        nc.sbuf_tensor("kbufs", [128, KT * WU], E4) as kbufs,
        nc.sbuf_tensor("out_sb", [128, 2 * M_PER], mybir.dt.bfloat16) as out_sb,
        nc.psum_tensor("acc", [128, 8 * 512], mybir.dt.float32) as acc,
        nc.semaphore("bank_sem") as bank_sem,
        nc.semaphore("dve_done") as dve_done,
        nc.semaphore("act_done") as act_done,
        nc.semaphore("out_sem") as out_sem,
        nc.Block(no_gpsimd_drain=True) as block,
    ):
        dma_sems = [stack.enter_context(nc.semaphore(f"dma_sem{i}"))
                    for i in range(NSEM)]

        @block.sync
        def _(sync):
            # no ring reuse: issue every batch back-to-back on the sync
            # HWDGE ring; the DGE drains them in order as one continuous
            # stream. (Splitting batches across both rings regresses:
            # engines round-robin rings at packet granularity, breaking
            # in-order tile delivery.) Exception: batch 0's second tile
            # rides the otherwise-idle scalar ring so the first pair
            # lands ~2x sooner and its completion posts on a ring with
            # no descriptor backlog; PE waits sem0 >= 32 for the pair.
            sync.dma_start(
                out=kbufs[:, :WU],
                in_=atf[:, :WU],
            ).then_inc(dma_sems[0], 16)
            for bt in range(1, NB):
                if bt >= 5:
                    # keep the descriptor ring ~4 batches deep: a deep
                    # backlog delays completion posting by 1.5-3us,
                    # which is what actually gates the PE early on.
                    pb = bt - 4
                    sync.wait_ge(dma_sems[pb % NSEM],
                                 16 * (pb // NSEM + 1)
                                 + (16 if pb % NSEM == 0 else 0))
                off = _TSTART[bt] * WU
                sz = BATCH[bt] * WU
                sync.dma_start(
                    out=kbufs[:, off:off + sz],
                    in_=atf[:, off:off + sz],
                ).then_inc(dma_sems[bt % NSEM], 16)
            # stores: chains evacuate on alternating DVE/ACT (ACT: 0,2,4;
            # DVE: 1,3,5) so every cast lands within ~1us of the last
            # matmul; 4-way split so early chunks stream while later
            # chains still cast (sync: cc0 halves, scalar: cc1 halves).
            sync.wait_ge(act_done, 1)         # chain 0
            sync.dma_start(
                out=out[:128, :MCH],
                in_=out_sb[:, :MCH],
            ).then_inc(out_sem, 16)
            sync.wait_ge(act_done, 2)         # chain 2
            sync.wait_ge(dve_done, 1)         # chain 1
            sync.dma_start(
                out=out[:128, MCH:],
                in_=out_sb[:, MCH:M_PER],
            ).then_inc(out_sem, 16)
            sync.wait_ge(dve_done, 3)         # chain 5 — final 98KB chunk
            sync.dma_start(
                out=out[128:, 2 * MCH:],
                in_=out_sb[:, M_PER + 2 * MCH:],
            ).then_inc(out_sem, 16)
            sync.wait_ge(out_sem, 80)

        @block.tensor
        def _(tensor):
            # p-state warm-up: burn the runtime-startup window with junk
            # matmuls into the spare PSUM bank so the clock ramp is done
            # by the time batch 0 lands. Reads uninitialized SBUF.
            wpair = kbufs[:, M_PER:M_PER + 2 * C].rearrange(
                "p (two c) -> p two c", two=2)
            wrhs = (kbufs[:, 0:MCH].unsqueeze(1)
                    .broadcast_to([128, 2, MCH]))
            for _ in range(NDUMMY):
                tensor.matmul(acc[:, 6 * 512:6 * 512 + MCH],
                              wpair[:, :, 0:128], wrhs,
                              start=True, stop=True, perf_mode=DR)

            for bt in range(NB):
                # sem0 carries an extra +16: batch 0 is two dma_starts
                thresh = 16 * (bt // NSEM + 1) + (16 if bt % NSEM == 0 else 0)
                tensor.wait_ge(dma_sems[bt % NSEM], thresh)
                for sp in range(_TSTART[bt] // 2,
                                (_TSTART[bt] + BATCH[bt]) // 2):
                    base = 2 * sp * WU
                    pair = kbufs[:, base:base + 2 * WU].rearrange(
                        "p (two w) -> p two w", two=2)
                    fin = sp == KT // 2 - 1
                    for cc in range(2):
                        lhsT = pair[:, :, M_PER + cc * 128:
                                    M_PER + (cc + 1) * 128]
                        for mm in range(3):
                            q = cc * 3 + mm
                            inst = tensor.matmul(
                                acc[:, q * 512:q * 512 + MCH],
                                lhsT,
                                pair[:, :, mm * MCH:(mm + 1) * MCH],
                                start=(sp == 0), stop=fin,
                                perf_mode=DR,
                            )
                            if mm > 0:
                                # same lhsT as mm=0: reuse the loaded
                                # weights, skip the redundant LDWEIGHTS
                                inst.ins.ldweights = False
                            if fin:
                                inst.then_inc(bank_sem, 1)

        @block.vector
        def _(vector):
            # DVE evacuates odd chains 1, 3, 5
            for q in (1, 3, 5):
                vector.wait_ge(bank_sem, q + 1)
                dst = (q // 3) * M_PER + (q % 3) * MCH
                vector.tensor_copy(
                    out_sb[:, dst:dst + MCH],
                    acc[:, q * 512:q * 512 + MCH]).then_inc(dve_done, 1)

        @block.scalar
        def _(scalar):
            # batch 0's second tile on the scalar HWDGE ring (see sync)
            scalar.dma_start(
                out=kbufs[:, WU:2 * WU],
                in_=atf[:, WU:2 * WU],
            ).then_inc(dma_sems[0], 16)
            # Warm the ACT table off the critical tail.
            scalar.copy(out_sb[:1, :1], out_sb[:1, :1])
            # ACT evacuates even chains 0, 2, 4
            for q in (0, 2):
                scalar.wait_ge(bank_sem, q + 1)
                scalar.copy(out_sb[:, q * MCH:(q + 1) * MCH],
                            acc[:, q * 512:q * 512 + MCH]).then_inc(act_done, 1)
            scalar.wait_ge(bank_sem, 5)
            scalar.copy(out_sb[:, M_PER + MCH:M_PER + 2 * MCH],
                        acc[:, 4 * 512:4 * 512 + MCH])
            scalar.wait_ge(dve_done, 2)       # chain 3
            scalar.dma_start(
                out=out[128:, :MCH],
                in_=out_sb[:, M_PER:M_PER + MCH],
            ).then_inc(out_sem, 16)
            # chain 4 was cast by us just above (same-engine ordering)
            scalar.dma_start(
                out=out[128:, MCH:2 * MCH],
                in_=out_sb[:, M_PER + MCH:M_PER + 2 * MCH],
            ).then_inc(out_sem, 16)

    return nc


def prep_inputs(weights: np.ndarray, cnn_feature: np.ndarray):
    """Quantize + pack per-core e4m3 partition-major images; return
    (in_maps, scales, rank-1 colsum term)."""
    import ml_dtypes
    e4np = ml_dtypes.float8_e4m3

    A = np.asarray(weights, dtype=np.float32).reshape(HW, HW)
    F = np.asarray(cnn_feature, dtype=np.float32).reshape(C, HW).T  # [HW, C]

    s_F = np.float32(240.0) / np.float32(np.abs(F).max())
    F8 = (F * s_F).astype(e4np)
    F8t = F8.reshape(KT, 128, C)

    colsum = np.float64(0.5) * F.astype(np.float64).sum(axis=0)

    u = A - np.float32(0.5)
    in_maps = []
    scales = []
    for i in range(N_CORES):
        ush = u[i * M_PER:(i + 1) * M_PER, :]
        s_u = np.float32(240.0) / np.float32(np.abs(ush).max())
        u8t = np.ascontiguousarray(ush.T * s_u).astype(e4np)   # [HW, 1152]
        # partition-major pack: atf[p, j*WU:(j+1)*WU] = [u8t | F8] of
        # k-row j*128+p
        atf = np.concatenate(
            [u8t.reshape(KT, 128, M_PER), F8t], axis=2
        ).transpose(1, 0, 2).reshape(128, KT * WU)
        in_maps.append({"atf": np.ascontiguousarray(atf)})
        scales.append(float(s_u) * float(s_F))
    return in_maps, scales, colsum


def kernel(weights: np.ndarray, cnn_feature: np.ndarray) -> np.ndarray:
    in_maps, scales, colsum = prep_inputs(weights, cnn_feature)
    nc = build_bass()
    res = run_bass_kernel_spmd(nc, in_maps, list(range(N_CORES)))
    parts = []
    for i in range(N_CORES):
        raw = np.asarray(res.results[i]["out"]).astype(np.float32)
        parts.append(raw.T.astype(np.float64) / scales[i] + colsum[None, :])
    full = np.concatenate(parts, axis=0).astype(np.float32)
    return full.reshape(HW, 1, 1, C)
